# revision 60
# baseline (speedup 1.0000x reference)
"""Trainium2 Bass kernel for the CSMHP (clustered self-exciting Hawkes process)
negative log-likelihood, distributed over 8 NeuronCores.

Math
----
The reference builds the full (C, N, N) pairwise decay tensor and row-reduces
it with logsumexp.  The excitation

    E[c, i] = sum_{j<i} exp(-beta_c * (t_i - t_j))

obeys the first-order recurrence  E_i = d_i * (E_{i-1} + 1)  with
d_i = exp(-beta_c * (t_i - t_{i-1})), which maps exactly onto the DVE
`tensor_tensor_scan` instruction: state = (d *mult* state) *add* d.
That turns the O(N^2 C) pairwise tensor into O(N C) work.

Sharding
--------
Events are split into 8 contiguous blocks of 512 (the N axis of the pairwise
tensor, as the hint suggests).  Each core:
  * computes its scan-initial state A'[c] = E[c, first_own_event - 1] directly
    from the (padded, uniform-shape) list of prior events — a dense
    exp+reduce over at most 3584 values, so no cross-core recurrence and no
    collectives are needed;
  * runs the scan over its 512 events for all 8 clusters at once;
  * reduces its partial log-likelihood sum, its partial probability column
    sum, and (core 7) the excitation at the very last event, which is exactly
    the data the analytic integral term needs.
The host sums the 8 partial scalars (the "all-reduce" of the hint).
"""

import numpy as np

import concourse.bass as bass
from concourse import mybir
from concourse.bass_utils import run_bass_kernel_spmd

F32 = mybir.dt.float32
ALU = mybir.AluOpType
ACT = mybir.ActivationFunctionType

N = 4096
C = 8
NCORES = 8
CHUNK = N // NCORES          # 512 events per core
PRIOR_PAD = 3584             # max prior events (core 7: 512*7-1=3583), padded
PCOLS = PRIOR_PAD // 128     # 28
T_WINDOW = 100.0
BIG = 1.0e9                  # pad offset: exp(-beta*BIG) == 0 in fp32

_NC_CACHE = None


def _build_nc(with_clears: bool = True):
    """Raw-Bass SPMD program, single basic block, hand-placed semaphores.

    All engines walk the same basic-block graph, so keeping every
    instruction in the entry block means no engine ever takes a branch
    (branch targets outside the current IRAM block stall ~2-3us on the
    ifetch DMA).  Instruction order within the block gives each engine its
    queue order.

    Chains: the inB-fed prolog (wbig -> exp -> reduce -> matmul) and the
    inA-fed decay chain (dt -> exp) run on separate input semaphores and
    converge at the scan, which reads the history matmul's PSUM directly.
    The intensity product, matmul, and Ln run in two pipelined halves on
    separate PSUM banks.

    Same-engine RAW pairs are separated by an intervening wait_ge (the
    queue stall drains the non-interlocked pipeline) or a sem handshake.
    """
    nc = bass.Bass("TRN2", target_bir_lowering=False, debug=False)

    # inA columns: [0:512] t_own | [512:1024] t_prev | [1024:1536] pT
    #              [1536:1540] scal (beta, alpha, mu, gamma)
    ina_d = nc.dram_tensor("inA", [C, 3 * CHUNK + 4], F32, kind="ExternalInput")
    # inB columns: [0:224] prior_rep | [224:225] tref | [225:449] b128_rep
    #              [449:457] ones | [457:458] zeros
    CP = C * PCOLS
    inb_d = nc.dram_tensor("inB", [128, 2 * CP + 2 + C], F32, kind="ExternalInput")
    # out columns: 0 = probability column sums, 1 = last-event excitation,
    # 2,3 = ll partials (half sums; all rows identical)
    out_d = nc.dram_tensor("out", [C, 4], F32, kind="ExternalOutput")

    H = CHUNK // 2  # 256

    from contextlib import ExitStack

    ctx = ExitStack()
    sb = lambda name, shape: ctx.enter_context(nc.sbuf_tensor(name, shape, F32))
    psum = lambda name, shape: ctx.enter_context(
        nc.psum_tensor(name, shape, F32)
    )
    sem = lambda name: ctx.enter_context(nc.semaphore(name))
    with ctx:
        ina = sb("ina", [C, 3 * CHUNK + 4])
        inb = sb("inb", [128, 2 * CP + 2 + C])
        wbig = sb("wbig", [128, CP])
        ebig = sb("ebig", [128, C, PCOLS])
        r_part = sb("r_part", [128, C])
        dt2 = sb("dt2", [C, CHUNK])
        dec = sb("dec", [C, CHUNK])
        base = sb("base", [C, CHUNK])
        exc = sb("exc", [C, CHUNK])
        lamb = sb("lamb", [C, CHUNK])
        pl = ctx.enter_context(
            nc.sbuf_tensor("pl", [C, CHUNK], mybir.dt.float32r)
        )
        ones8r = ctx.enter_context(
            nc.sbuf_tensor("ones8r", [C, C], mybir.dt.float32r)
        )
        logi = sb("logi", [C, CHUNK])
        out_stage = sb("out_stage", [C, 4])
        a_init = psum("a_init", [C, 1])
        inten1 = psum("inten1", [C, H])
        inten2 = psum("inten2", [C, H])
        s_ina = sem("s_ina")
        s_inb = sem("s_inb")
        s_dve = sem("s_dve")
        s_act = sem("s_act")
        s_pe = sem("s_pe")
        s_stage = sem("s_stage")
        s_out = sem("s_out")
        s_v = sem("s_v")
        s_pool = sem("s_pool")

        ina_ap = ina.ap()
        t_own = ina_ap[:, 0:CHUNK]
        t_prev = ina_ap[:, CHUNK : 2 * CHUNK]
        pt = ina_ap[:, 2 * CHUNK : 3 * CHUNK]
        scal = ina_ap[:, 3 * CHUNK : 3 * CHUNK + 4]
        inb_ap = inb.ap()
        prior_rep = inb_ap[:, 0:CP]
        tref = inb_ap[:, CP : CP + 1]
        b128_rep = inb_ap[:, CP + 1 : 2 * CP + 1]
        ones_in = inb_ap[:, 2 * CP + 1 : 2 * CP + 1 + C]
        zeros128 = inb_ap[:, 2 * CP + 1 + C : 2 * CP + 2 + C]

        beta_col = scal[:, 0:1]
        alpha_col = scal[:, 1:2]
        mu_col = scal[:, 2:3]
        gammat_col = scal[:, 3:4]

        # record the length of the framework-emitted prefix; only that
        # prefix is eligible for scaffolding stripping
        n_prefix = len(nc.m.functions[0].blocks[0].instructions)

        # ---- input DMA issue: one inB partition-slice per idle engine so
        # the four transfers ride four DGE queues in parallel (a single
        # queue sustains only ~50 GB/s); inA via Pool, which then computes
        # base = mu + (gamma/T)*t off the DVE critical path ----
        nc.scalar.dma_start(
            out=inb.ap()[0:64, :], in_=inb_d.ap()[0:64, :]
        ).then_inc(s_inb, 16)
        nc.sync.dma_start(
            out=inb.ap()[64:128, :], in_=inb_d.ap()[64:128, :]
        ).then_inc(s_inb, 16)
        nc.gpsimd.dma_start(out=ina.ap(), in_=ina_d.ap()).then_inc(s_ina, 16)
        nc.gpsimd.wait_ge(s_ina, 16)
        nc.gpsimd.tensor_scalar(
            out=base.ap(), in0=t_own, scalar1=gammat_col, scalar2=mu_col,
            op0=ALU.mult, op1=ALU.add,
        ).then_inc(s_pool, 1)                                      # s_pool 1

        # ---- DVE stream ----
        nc.vector.wait_ge(s_inb, 32)
        nc.vector.scalar_tensor_tensor(
            out=wbig.ap(), in0=prior_rep, scalar=tref, in1=b128_rep,
            op0=ALU.subtract, op1=ALU.mult,
        ).then_inc(s_dve, 1)                                       # s_dve 1
        nc.vector.tensor_copy(ones8r.ap(), ones_in[0:C, :])
        nc.vector.wait_ge(s_ina, 16)
        nc.vector.tensor_sub(dt2.ap(), t_prev, t_own).then_inc(
            s_dve, 1
        )                                                          # s_dve 2
        nc.vector.reduce_sum(
            out_stage.ap()[:, 0:1], pt, axis=mybir.AxisListType.X
        ).then_inc(s_stage, 1)                                     # s_stage 1
        nc.vector.wait_ge(s_act, 1)        # ebig exp done
        nc.vector.reduce_sum(
            r_part.ap(), ebig.ap(), axis=mybir.AxisListType.X
        ).then_inc(s_dve, 1)                                       # s_dve 3
        nc.vector.wait_ge(s_act, 2)        # dec exp done
        nc.vector.wait_ge(s_pe, 1)         # history matmul done
        nc.vector.tensor_tensor_scan(
            exc.ap(), dec.ap(), dec.ap(), initial=a_init.ap(),
            op0=ALU.mult, op1=ALU.add,
        ).then_inc(s_v, 1)                                         # s_v 1
        nc.vector.wait_ge(s_v, 1)
        nc.vector.wait_ge(s_pool, 1)       # base ready (Pool)
        nc.vector.scalar_tensor_tensor(
            out=lamb.ap()[:, 0:H], in0=exc.ap()[:, 0:H], scalar=alpha_col,
            in1=base.ap()[:, 0:H], op0=ALU.mult, op1=ALU.add,
        ).then_inc(s_v, 1)                                         # s_v 2
        nc.vector.wait_ge(s_v, 2)
        nc.vector.tensor_mul(
            pl.ap()[:, 0:H], lamb.ap()[:, 0:H], pt[:, 0:H]
        ).then_inc(s_dve, 1)                                       # s_dve 4
        nc.vector.scalar_tensor_tensor(
            out=lamb.ap()[:, H:CHUNK], in0=exc.ap()[:, H:CHUNK],
            scalar=alpha_col, in1=base.ap()[:, H:CHUNK],
            op0=ALU.mult, op1=ALU.add,
        ).then_inc(s_v, 1)                                         # s_v 3
        nc.vector.wait_ge(s_v, 3)
        nc.vector.tensor_mul(
            pl.ap()[:, H:CHUNK], lamb.ap()[:, H:CHUNK], pt[:, H:CHUNK]
        ).then_inc(s_dve, 1)                                       # s_dve 5
        nc.vector.tensor_copy(
            out_stage.ap()[:, 1:2], exc.ap()[:, CHUNK - 1 : CHUNK]
        ).then_inc(s_stage, 1)                                     # s_stage 1

        # ---- ACT stream ----
        nc.scalar.wait_ge(s_inb, 32)
        nc.scalar.wait_ge(s_dve, 1)
        nc.scalar.activation(
            ebig.ap(), wbig.ap().rearrange("p (c f) -> p c f", c=C),
            ACT.Exp, bias=zeros128,
        ).then_inc(s_act, 1)                                       # s_act 1
        nc.scalar.wait_ge(s_ina, 16)
        nc.scalar.wait_ge(s_dve, 2)
        nc.scalar.activation(
            dec.ap(), dt2.ap(), ACT.Exp, bias=zeros128[0:C, :],
            scale=beta_col,
        ).then_inc(s_act, 1)                                       # s_act 2
        nc.scalar.wait_ge(s_pe, 2)
        nc.scalar.activation(
            logi.ap()[:, 0:H], inten1.ap(), ACT.Ln,
            bias=zeros128[0:C, :], accum_out=out_stage.ap()[:, 2:3],
        ).then_inc(s_act, 1)                                       # s_act 3
        nc.scalar.wait_ge(s_pe, 3)
        nc.scalar.activation(
            logi.ap()[:, H:CHUNK], inten2.ap(), ACT.Ln,
            bias=zeros128[0:C, :], accum_out=out_stage.ap()[:, 3:4],
        ).then_inc(s_act, 1)                                       # s_act 4
        nc.scalar.wait_ge(s_stage, 2)
        nc.scalar.wait_ge(s_act, 4)
        nc.scalar.dma_start(out=out_d.ap(), in_=out_stage.ap()).then_inc(
            s_out, 16
        )

        # ---- PE stream ----
        nc.tensor.wait_ge(s_inb, 32)
        nc.tensor.wait_ge(s_dve, 3)
        nc.tensor.matmul(
            a_init.ap(), r_part.ap(), ones_in[:, 0:1], start=True, stop=True
        ).then_inc(s_pe, 1)                                        # s_pe 1
        # float32r: 1 cycle/row at moving dim >= 256 (4x over fp32); the
        # producers write fp32r-rounded values as the verifier requires
        nc.tensor.wait_ge(s_dve, 4)
        nc.tensor.matmul(
            inten1.ap(), ones8r.ap(), pl.ap()[:, 0:H],
            start=True, stop=True,
        ).then_inc(s_pe, 1)                                        # s_pe 2
        nc.tensor.wait_ge(s_dve, 5)
        nc.tensor.matmul(
            inten2.ap(), ones8r.ap(), pl.ap()[:, H:CHUNK],
            start=True, stop=True,
        ).then_inc(s_pe, 1)                                        # s_pe 3

        # ---- SP stream: track completion + semaphore reset ----
        nc.sync.wait_ge(s_out, 16)
        if with_clears:
            # every other stream has retired here (transitively through
            # s_out); reset all eight sems in one range-clear for the next
            # execution of the same loaded NEFF
            nc.sync.wait_ge(s_ina, 16)
            nc.sync.wait_ge(s_inb, 16)
            sem_ids = sorted(
                s.num
                for s in (s_ina, s_inb, s_dve, s_act, s_pe, s_stage, s_out,
                          s_v, s_pool)
            )
            assert sem_ids[-1] - sem_ids[0] == 8, sem_ids
            nc.sync.sem_clear(range(sem_ids[0], sem_ids[-1] + 1))

    _strip_entry_scaffolding(nc, n_prefix)
    return nc


def _strip_entry_scaffolding(nc, n_prefix):
    """Remove the const-AP Pool memsets and the Bass.__init__ all-engine
    barrier from the entry block.  No instruction in this program reads the
    const APs (activation biases are explicit inB columns), and the
    inter-execution fence the barrier provides is already guaranteed by the
    runtime (execution N+1 starts only after N's queues fully retire).
    Pool's dge_drain in that barrier otherwise delays the input DMAs by
    ~3.5us."""
    main = nc.m.functions[0].blocks[0]
    drop_types = ("InstMemset", "InstDrain", "InstEventSemaphore")
    kept = [
        inst
        for i, inst in enumerate(main.instructions)
        if i >= n_prefix or type(inst).__name__ not in drop_types
    ]
    main.instructions[:] = kept


def get_nc(with_clears: bool = True):
    global _NC_CACHE
    if _NC_CACHE is None:
        _NC_CACHE = _build_nc(with_clears)
    return _NC_CACHE


def make_in_maps(probability, event_times, mu, gamma, alpha_kernel, beta_kernel):
    t = np.ascontiguousarray(np.asarray(event_times, dtype=np.float32))
    p = np.ascontiguousarray(np.asarray(probability, dtype=np.float32))
    beta = np.asarray(beta_kernel, dtype=np.float32)
    alpha = np.asarray(alpha_kernel, dtype=np.float32)
    mu_ = np.asarray(mu, dtype=np.float32)
    gamma_ = np.asarray(gamma, dtype=np.float32)

    scal = np.stack([beta, alpha, mu_, gamma_ / np.float32(T_WINDOW)], axis=1)
    b128 = np.broadcast_to(beta, (128, C))

    in_maps = []
    for k in range(NCORES):
        s = k * CHUNK
        t_own = np.broadcast_to(t[s : s + CHUNK], (C, CHUNK))
        tp = np.empty(CHUNK, np.float32)
        if k == 0:
            tp[0] = t[0] - BIG  # forces d_0 = 0: no events precede event 0
            tp[1:] = t[: CHUNK - 1]
        else:
            tp[:] = t[s - 1 : s + CHUNK - 1]
        t_prev = np.broadcast_to(tp, (C, CHUNK))
        pt = p[s : s + CHUNK, :].T

        npri = max(s - 1, 0)
        pri = np.full(PRIOR_PAD, -BIG, np.float32)
        pri[:npri] = t[:npri]
        prior_pm = pri.reshape(PCOLS, 128).T
        tref_val = t[s - 1] if k > 0 else t[0]
        tref = np.full((128, 1), tref_val, np.float32)

        ina = np.ascontiguousarray(
            np.concatenate([t_own, t_prev, pt, scal], axis=1, dtype=np.float32)
        )
        ones_c = np.ones((128, C), np.float32)
        prior_rep = np.tile(prior_pm, (1, C))                       # (128, 224)
        b128_rep = np.broadcast_to(
            np.repeat(beta, PCOLS)[None, :], (128, C * PCOLS)
        )
        zeros_c = np.zeros((128, 1), np.float32)
        inb = np.ascontiguousarray(
            np.concatenate(
                [prior_rep, tref, b128_rep, ones_c, zeros_c],
                axis=1, dtype=np.float32,
            )
        )
        in_maps.append({"inA": ina, "inB": inb})
    return in_maps


def combine_outputs(results, event_times, mu, gamma, alpha_kernel, beta_kernel):
    """Host-side reduction of the per-core partial scalars (float64)."""
    t = np.asarray(event_times, dtype=np.float32)
    beta = np.asarray(beta_kernel, dtype=np.float64)
    alpha = np.asarray(alpha_kernel, dtype=np.float64)
    mu_ = np.asarray(mu, dtype=np.float64)
    gamma_ = np.asarray(gamma, dtype=np.float64)

    ll_sum = sum(float(r["out"][0, 2]) + float(r["out"][0, 3]) for r in results)
    psum = np.zeros(C, np.float64)
    for r in results:
        psum += r["out"][:, 0].astype(np.float64)
    elast = results[NCORES - 1]["out"][:, 1].astype(np.float64)

    ab = alpha / beta
    exp_term = ab * ((N - 1) - elast)
    t_diff = float(t[-1]) - float(t[0])
    t_sq_diff = float(t[-1]) ** 2 - float(t[0]) ** 2
    base_terms = t_diff * mu_ + t_sq_diff * gamma_ / (2.0 * T_WINDOW)
    integral_part = float(psum @ (exp_term + base_terms)) / N
    return np.float32(-(ll_sum - integral_part))


def kernel(probability, event_times, mu, gamma, alpha_kernel, beta_kernel):
    nc = get_nc()
    in_maps = make_in_maps(
        probability, event_times, mu, gamma, alpha_kernel, beta_kernel
    )
    res = run_bass_kernel_spmd(nc, in_maps, core_ids=list(range(NCORES))).results
    return combine_outputs(
        res, event_times, mu, gamma, alpha_kernel, beta_kernel
    )
